# revision 16
# baseline (speedup 1.0000x reference)
"""BitConv2d forward on 8 Trainium2 NeuronCores (SPMD data-parallel).

Strategy:
  - Shard batch (32) -> 4 images per core; replicate the tiny bit-plane
    weights/scales on every core. No collectives needed (forward only).
  - Host precomputes the integer conv weights
        W_int[o,i,kh,kw] = sum_b (pweight-nweight)[...,b] * 2^(3-b)   (exact, in [-15,15])
    plus fused scale (scale/15) and bias vectors; bf16 is exact for
    ints <= 15, so the only precision loss is x (bf16) and the fp16
    output store (~1.6e-3 max-rel combined).
  - Host pre-pads each image into the SBUF layout the matmuls read
    ([128, 59*114] bf16: partitions 0:64 = half A = padded rows 0..57,
    partitions 64:128 = half B = padded rows 55..112+zero, row-flattened
    at stride 114, zero pad columns included).
  - On device each half is staged TWICE per image: tile xpA = [half A;
    half A shifted left 1 col], tile xpB likewise for half B. A 3x3 conv
    tap pair (kh,0)+(kh,1) then becomes ONE K=128 matmul against xpA
    (top 64 contraction rows = first tap, bottom 64 = second tap), and
    the two halves' matmuls run CONCURRENTLY as M=64 column-group tiles
    of the 128x128 PE array (tile_position (0,0) / (0,64) - measured
    ~3ns stagger). 9 taps/tile collapse to 6 matmul slots: 3 tap-pair
    slots + 3 single-tap slots (kw=2 column, bottom rows zero-weighted).
  - Epilogue on ACT: out = psum*(scale/15) + bias, cast to fp16; stores
    stream out per 512-col tile. All DMAs are issued in ~1KB-per-
    partition column chunks so descriptors rotate across partitions
    (per-partition SBUF port serializes big descriptors; tiny 448B ones
    are descriptor-rate-bound). Loads ride the gpsimd + vector queues,
    stores the sync queue.
  - Host reassembles the raw [128, 56*114] fp16 tiles to NCHW f32.
"""

import numpy as np
import ml_dtypes

B, C, H, W = 32, 64, 112, 112
NB = 4
CORES = 8
BPC = B // CORES  # images per core

WP = H + 2  # padded width/height = 114
HALF = H // 2  # 56 output rows per position-group
XC_DATA = 58 * WP  # 6612 data columns per partition block
XC = 59 * WP  # + one zero row (junk-column tap reads run past the data)
OUTC = HALF * WP  # 6384 output columns per group

# N-tiles: all >=256 for full PE rate (PSUM bank caps at 512 fp32)
N_TILES = [(i * 512, 512) for i in range(11)] + [(5632, 376), (6008, 376)]
# 6 matmul slots: (rhs column offset, top-tap, bottom-tap or None)
# taps t=(kh,kw) offset kh*114+kw; bottom rows read the shifted copy (+1 col)
SLOTS = [
    (0, 0, 1),      # pair (0,0)+(0,1)
    (114, 3, 4),    # pair (1,0)+(1,1)
    (228, 6, 7),    # pair (2,0)+(2,1)
    (2, 2, None),   # single (0,2)
    (116, 5, None), # single (1,2)
    (230, 8, None), # single (2,2)
]

LOAD_CHUNK = 512  # elements per DMA chunk (~1KB/partition descriptors)

_CACHE = {}


def _build():
    if "nc" in _CACHE:
        return _CACHE["nc"]
    import concourse.bacc as bacc
    import concourse.mybir as mybir
    from concourse import tile

    f32 = mybir.dt.float32
    f16 = mybir.dt.float16
    bf16 = mybir.dt.bfloat16

    nc = bacc.Bacc("TRN2", target_bir_lowering=False, debug=False, num_devices=CORES)

    xp_d = nc.dram_tensor("xp", [BPC, 128, XC], bf16, kind="ExternalInput").ap()
    wp_d = nc.dram_tensor("wp6", [128, 6 * C], bf16, kind="ExternalInput").ap()
    sc_d = nc.dram_tensor("scalev", [128, 1], f32, kind="ExternalInput").ap()
    bi_d = nc.dram_tensor("biasv", [128, 1], f32, kind="ExternalInput").ap()
    y_d = nc.dram_tensor("y", [BPC, 128, OUTC], f16, kind="ExternalOutput").ap()

    with tile.TileContext(nc) as tc:
        with (
            tc.tile_pool(name="consts", bufs=1) as consts,
            tc.tile_pool(name="xpool", bufs=3) as xpool,
            tc.tile_pool(name="opool", bufs=2) as opool,
            tc.tile_pool(name="pspool", bufs=8, space="PSUM") as pspool,
        ):
            wp6 = consts.tile([128, 6 * C], bf16, tag="wp6")
            nc.sync.dma_start(wp6[:], wp_d)
            scale_vec = consts.tile([128, 1], f32, tag="scale_vec")
            bias_vec = consts.tile([128, 1], f32, tag="bias_vec")
            nc.sync.dma_start(scale_vec[:], sc_d)
            nc.sync.dma_start(bias_vec[:], bi_d)

            # ---- image load pipeline ----
            # xpA = [half, half shifted left 1]; chunked so descriptors stay
            # ~1KB and rotate partitions. half A rides gpsimd queue, half B
            # the vector queue.
            def load_image(b):
                tiles = []
                for name, eng, p0 in (("xpA", nc.gpsimd, 0), ("xpB", nc.scalar, C)):
                    xt = xpool.tile([128, XC], bf16, tag=name, name=f"{name}{b}")
                    src = xp_d[b, p0 : p0 + C]
                    for c0 in range(0, XC, LOAD_CHUNK):
                        c1 = min(c0 + LOAD_CHUNK, XC)
                        eng.dma_start(xt[0:C, c0:c1], src[:, c0:c1])
                        c1s = min(c1, XC - 1)
                        if c0 < c1s:
                            eng.dma_start(
                                xt[C:128, c0:c1s], src[:, c0 + 1 : c1s + 1]
                            )
                    tiles.append(xt)
                return tiles

            img_next = load_image(0)
            img_next2 = load_image(1)

            # ---- main conv loop ----
            for b in range(BPC):
                xpA, xpB = img_next
                img_next = img_next2
                img_next2 = load_image(b + 2) if b + 2 < BPC else None

                outb = opool.tile([128, OUTC], f16, tag="outb")
                for n0, nt in N_TILES:
                    ps = pspool.tile([128, 512], f32, tag="ps")
                    for s, (off, _t, _tb) in enumerate(SLOTS):
                        w_s = wp6[:, s * C : (s + 1) * C]
                        nc.tensor.matmul(
                            ps[0:C, 0:nt],
                            w_s,
                            xpA[:, n0 + off : n0 + off + nt],
                            start=(s == 0),
                            stop=(s == len(SLOTS) - 1),
                            tile_position=(0, 0),
                        )
                        nc.tensor.matmul(
                            ps[C:128, 0:nt],
                            w_s,
                            xpB[:, n0 + off : n0 + off + nt],
                            start=(s == 0),
                            stop=(s == len(SLOTS) - 1),
                            tile_position=(0, 64),
                        )
                    nc.scalar.activation(
                        outb[:, n0 : n0 + nt],
                        ps[:, 0:nt],
                        mybir.ActivationFunctionType.Identity,
                        bias=bias_vec[:],
                        scale=scale_vec[:],
                    )
                    nc.sync.dma_start(y_d[b, :, n0 : n0 + nt], outb[:, n0 : n0 + nt])

    nc.compile()
    _CACHE["nc"] = nc
    return nc


def _prep_inputs(inputs):
    x = np.asarray(inputs["x"], dtype=np.float32)
    pw = np.asarray(inputs["pweight"], np.float32)
    nw = np.asarray(inputs["nweight"], np.float32)
    pb = np.asarray(inputs["pbias"], np.float32)
    nb = np.asarray(inputs["nbias"], np.float32)
    scale = np.asarray(inputs["scale"], np.float32)[0]
    bscale = np.asarray(inputs["biasscale"], np.float32)[0]

    exps2 = np.array([8.0, 4.0, 2.0, 1.0], np.float32)
    wint = ((pw - nw) * exps2).sum(-1)  # [O, I, 3, 3], exact ints in [-15, 15]
    bias = ((pb - nb) * exps2).sum(-1) * (bscale / 15.0)  # [O]

    # slot-packed stationary operands: [k=128, slot, m=64]
    wp6 = np.zeros((128, 6, C), np.float32)
    for s, (_off, t_top, t_bot) in enumerate(SLOTS):
        kh, kw = divmod(t_top, 3)
        wp6[0:C, s] = wint[:, :, kh, kw].T
        if t_bot is not None:
            kh, kw = divmod(t_bot, 3)
            wp6[C:128, s] = wint[:, :, kh, kw].T
    wp6 = wp6.reshape(128, 6 * C).astype(ml_dtypes.bfloat16)

    scale_vec = np.full((128, 1), scale / 15.0, np.float32)
    bias_vec = np.concatenate([bias, bias]).reshape(128, 1).astype(np.float32)

    # padded SBUF image layout
    xpad = np.zeros((B, 128, XC), dtype=np.float32)
    v = xpad[:, :, :XC_DATA].reshape(B, 128, 58, WP)
    # half A: padded rows 0..57 hold image rows -1..56 (row r = image row r-1)
    v[:, 0:C, 1:58, 1 : 1 + W] = x[:, :, 0:57, :]
    # half B: rows 0..56 hold image rows 55..111, row 57 stays zero
    v[:, C:128, 0:57, 1 : 1 + W] = x[:, :, 55:112, :]
    xpad16 = xpad.astype(ml_dtypes.bfloat16)

    shared = {"wp6": wp6, "scalev": scale_vec, "biasv": bias_vec}
    return [
        dict(shared, xp=np.ascontiguousarray(xpad16[c * BPC : (c + 1) * BPC]))
        for c in range(CORES)
    ]


def _assemble(results):
    """Raw [BPC, 128, 6384] fp16 per core -> [B, 64, 112, 112] f32."""
    out = np.empty((B, C, H, W), dtype=np.float32)
    for c in range(CORES):
        raw = np.asarray(results[c]["y"], dtype=np.float32).reshape(
            BPC, 128, HALF, WP
        )
        out[c * BPC : (c + 1) * BPC, :, 0:HALF, :] = raw[:, 0:C, :, 0:W]
        out[c * BPC : (c + 1) * BPC, :, HALF:H, :] = raw[:, C:128, :, 0:W]
    return out


def _run(inputs, trace=False):
    from concourse.bass_utils import run_bass_kernel_spmd

    nc = _build()
    in_maps = _prep_inputs(inputs)
    last_err = None
    for attempt in range(3):
        try:
            res = run_bass_kernel_spmd(
                nc, in_maps, core_ids=list(range(CORES)), trace=trace
            )
            return _assemble(res.results), res.exec_time_ns
        except Exception as e:  # transient NRT_EXEC_UNIT_UNRECOVERABLE recovers on retry
            last_err = e
            import time

            time.sleep(10)
    raise last_err


def kernel(**inputs) -> np.ndarray:
    out, _ = _run(inputs)
    return out


# revision 18
# speedup vs baseline: 1.4424x; 1.4424x over previous
"""BitConv2d forward on 8 Trainium2 NeuronCores (SPMD data-parallel).

Strategy:
  - Shard batch (32) -> 4 images per core; replicate the tiny bit-plane
    weights/scales on every core. No collectives needed (forward only).
  - Host precomputes the integer conv weights
        W_int[o,i,kh,kw] = sum_b (pweight-nweight)[...,b] * 2^(3-b)   (exact, in [-15,15])
    plus fused scale (scale/15) and bias vectors; bf16 is exact for
    ints <= 15, so the only precision loss is x (bf16) and the fp16
    output store (~1.6e-3 max-rel combined).
  - Host pre-pads each image into the SBUF layout the matmuls read
    ([128, 59*114] bf16: partitions 0:64 = half A = padded rows 0..57,
    partitions 64:128 = half B = padded rows 55..112+zero, row-flattened
    at stride 114, zero pad columns included).
  - On device each half is staged TWICE per image: tile xpA = [half A;
    half A shifted left 1 col], tile xpB likewise for half B. A 3x3 conv
    tap pair (kh,0)+(kh,1) then becomes ONE K=128 matmul against xpA
    (top 64 contraction rows = first tap, bottom 64 = second tap), and
    the two halves' matmuls run CONCURRENTLY as M=64 column-group tiles
    of the 128x128 PE array (tile_position (0,0) / (0,64) - measured
    ~3ns stagger). 9 taps/tile collapse to 6 matmul slots: 3 tap-pair
    slots + 3 single-tap slots (kw=2 column, bottom rows zero-weighted).
  - Epilogue on ACT: out = psum*(scale/15) + bias, cast to fp16; stores
    stream out per 512-col tile. All DMAs are issued in ~1KB-per-
    partition column chunks so descriptors rotate across partitions
    (per-partition SBUF port serializes big descriptors; tiny 448B ones
    are descriptor-rate-bound). Loads ride the gpsimd + vector queues,
    stores the sync queue.
  - Host reassembles the raw [128, 56*114] fp16 tiles to NCHW f32.
"""

import numpy as np
import ml_dtypes

B, C, H, W = 32, 64, 112, 112
NB = 4
CORES = 8
BPC = B // CORES  # images per core

WP = H + 2  # padded width/height = 114
HALF = H // 2  # 56 output rows per position-group
XC_DATA = 58 * WP  # 6612 data columns per partition block
XC = 59 * WP  # + one zero row (junk-column tap reads run past the data)
OUTC = HALF * WP  # 6384 output columns per group

# N-tiles: all >=256 for full PE rate (PSUM bank caps at 512 fp32)
N_TILES = [(i * 512, 512) for i in range(11)] + [(5632, 376), (6008, 376)]
# 6 matmul slots: (rhs column offset, top-tap, bottom-tap or None)
# taps t=(kh,kw) offset kh*114+kw; bottom rows read the shifted copy (+1 col)
SLOTS = [
    (0, 0, 1),      # pair (0,0)+(0,1)
    (114, 3, 4),    # pair (1,0)+(1,1)
    (228, 6, 7),    # pair (2,0)+(2,1)
    (2, 2, None),   # single (0,2)
    (116, 5, None), # single (1,2)
    (230, 8, None), # single (2,2)
]

LOAD_CHUNK = 1024  # elements per DMA chunk (~2KB/partition descriptors)

_CACHE = {}


def _build():
    if "nc" in _CACHE:
        return _CACHE["nc"]
    import concourse.bacc as bacc
    import concourse.mybir as mybir
    from concourse import tile

    f32 = mybir.dt.float32
    f16 = mybir.dt.float16
    bf16 = mybir.dt.bfloat16

    nc = bacc.Bacc("TRN2", target_bir_lowering=False, debug=False, num_devices=CORES)

    xp_d = nc.dram_tensor("xp", [BPC, 128, XC], bf16, kind="ExternalInput").ap()
    wp_d = nc.dram_tensor("wp6", [128, 6 * C], bf16, kind="ExternalInput").ap()
    sc_d = nc.dram_tensor("scalev", [128, 1], f32, kind="ExternalInput").ap()
    bi_d = nc.dram_tensor("biasv", [128, 1], f32, kind="ExternalInput").ap()
    y_d = nc.dram_tensor("y", [BPC, 128, OUTC], f16, kind="ExternalOutput").ap()

    with tile.TileContext(nc) as tc:
        with (
            tc.tile_pool(name="consts", bufs=1) as consts,
            tc.tile_pool(name="xpool", bufs=3) as xpool,
            tc.tile_pool(name="opool", bufs=2) as opool,
            tc.tile_pool(name="pspool", bufs=8, space="PSUM") as pspool,
        ):
            wp6 = consts.tile([128, 6 * C], bf16, tag="wp6")
            nc.sync.dma_start(wp6[:], wp_d)
            scale_vec = consts.tile([128, 1], f32, tag="scale_vec")
            bias_vec = consts.tile([128, 1], f32, tag="bias_vec")
            nc.sync.dma_start(scale_vec[:], sc_d)
            nc.sync.dma_start(bias_vec[:], bi_d)

            # ---- image load pipeline ----
            # xpA = [half A, half A shifted left 1], xpB likewise for half B.
            # All loads ride the gpsimd queue (sync carries stores, scalar
            # stays free for ACT epilogues), interleaved so both tiles fill
            # in column lockstep and the matmuls can chase the chunks.
            def load_image(b):
                xA = xpool.tile([128, XC], bf16, tag="xpA", name=f"xpA{b}", bufs=4)
                xB = xpool.tile([128, XC], bf16, tag="xpB", name=f"xpB{b}", bufs=4)
                srcA = xp_d[b, 0:C]
                srcB = xp_d[b, C:128]
                for c0 in range(0, XC, LOAD_CHUNK):
                    c1 = min(c0 + LOAD_CHUNK, XC)
                    c1s = min(c1, XC - 1)
                    for xt, src in ((xA, srcA), (xB, srcB)):
                        nc.gpsimd.dma_start(xt[0:C, c0:c1], src[:, c0:c1])
                        if c0 < c1s:
                            nc.gpsimd.dma_start(
                                xt[C:128, c0:c1s], src[:, c0 + 1 : c1s + 1]
                            )
                return xA, xB

            img_next = load_image(0)
            img_next2 = load_image(1)

            # ---- main conv loop ----
            for b in range(BPC):
                xpA, xpB = img_next
                img_next = img_next2
                img_next2 = load_image(b + 2) if b + 2 < BPC else None

                outb = opool.tile([128, OUTC], f16, tag="outb")
                for n0, nt in N_TILES:
                    ps = pspool.tile([128, 512], f32, tag="ps")
                    for s, (off, _t, _tb) in enumerate(SLOTS):
                        w_s = wp6[:, s * C : (s + 1) * C]
                        nc.tensor.matmul(
                            ps[0:C, 0:nt],
                            w_s,
                            xpA[:, n0 + off : n0 + off + nt],
                            start=(s == 0),
                            stop=(s == len(SLOTS) - 1),
                            tile_position=(0, 0),
                        )
                        nc.tensor.matmul(
                            ps[C:128, 0:nt],
                            w_s,
                            xpB[:, n0 + off : n0 + off + nt],
                            start=(s == 0),
                            stop=(s == len(SLOTS) - 1),
                            tile_position=(0, 64),
                        )
                    nc.scalar.activation(
                        outb[:, n0 : n0 + nt],
                        ps[:, 0:nt],
                        mybir.ActivationFunctionType.Identity,
                        bias=bias_vec[:],
                        scale=scale_vec[:],
                    )
                    nc.sync.dma_start(y_d[b, :, n0 : n0 + nt], outb[:, n0 : n0 + nt])

    nc.compile()
    _CACHE["nc"] = nc
    return nc


def _prep_inputs(inputs):
    x = np.asarray(inputs["x"], dtype=np.float32)
    pw = np.asarray(inputs["pweight"], np.float32)
    nw = np.asarray(inputs["nweight"], np.float32)
    pb = np.asarray(inputs["pbias"], np.float32)
    nb = np.asarray(inputs["nbias"], np.float32)
    scale = np.asarray(inputs["scale"], np.float32)[0]
    bscale = np.asarray(inputs["biasscale"], np.float32)[0]

    exps2 = np.array([8.0, 4.0, 2.0, 1.0], np.float32)
    wint = ((pw - nw) * exps2).sum(-1)  # [O, I, 3, 3], exact ints in [-15, 15]
    bias = ((pb - nb) * exps2).sum(-1) * (bscale / 15.0)  # [O]

    # slot-packed stationary operands: [k=128, slot, m=64]
    wp6 = np.zeros((128, 6, C), np.float32)
    for s, (_off, t_top, t_bot) in enumerate(SLOTS):
        kh, kw = divmod(t_top, 3)
        wp6[0:C, s] = wint[:, :, kh, kw].T
        if t_bot is not None:
            kh, kw = divmod(t_bot, 3)
            wp6[C:128, s] = wint[:, :, kh, kw].T
    wp6 = wp6.reshape(128, 6 * C).astype(ml_dtypes.bfloat16)

    scale_vec = np.full((128, 1), scale / 15.0, np.float32)
    bias_vec = np.concatenate([bias, bias]).reshape(128, 1).astype(np.float32)

    # padded SBUF image layout
    xpad = np.zeros((B, 128, XC), dtype=np.float32)
    v = xpad[:, :, :XC_DATA].reshape(B, 128, 58, WP)
    # half A: padded rows 0..57 hold image rows -1..56 (row r = image row r-1)
    v[:, 0:C, 1:58, 1 : 1 + W] = x[:, :, 0:57, :]
    # half B: rows 0..56 hold image rows 55..111, row 57 stays zero
    v[:, C:128, 0:57, 1 : 1 + W] = x[:, :, 55:112, :]
    xpad16 = xpad.astype(ml_dtypes.bfloat16)

    shared = {"wp6": wp6, "scalev": scale_vec, "biasv": bias_vec}
    return [
        dict(shared, xp=np.ascontiguousarray(xpad16[c * BPC : (c + 1) * BPC]))
        for c in range(CORES)
    ]


def _assemble(results):
    """Raw [BPC, 128, 6384] fp16 per core -> [B, 64, 112, 112] f32."""
    out = np.empty((B, C, H, W), dtype=np.float32)
    for c in range(CORES):
        raw = np.asarray(results[c]["y"], dtype=np.float32).reshape(
            BPC, 128, HALF, WP
        )
        out[c * BPC : (c + 1) * BPC, :, 0:HALF, :] = raw[:, 0:C, :, 0:W]
        out[c * BPC : (c + 1) * BPC, :, HALF:H, :] = raw[:, C:128, :, 0:W]
    return out


def _run(inputs, trace=False):
    from concourse.bass_utils import run_bass_kernel_spmd

    nc = _build()
    in_maps = _prep_inputs(inputs)
    last_err = None
    for attempt in range(3):
        try:
            res = run_bass_kernel_spmd(
                nc, in_maps, core_ids=list(range(CORES)), trace=trace
            )
            return _assemble(res.results), res.exec_time_ns
        except Exception as e:  # transient NRT_EXEC_UNIT_UNRECOVERABLE recovers on retry
            last_err = e
            import time

            time.sleep(10)
    raise last_err


def kernel(**inputs) -> np.ndarray:
    out, _ = _run(inputs)
    return out


# revision 20
# speedup vs baseline: 1.5225x; 1.0555x over previous
"""BitConv2d forward on 8 Trainium2 NeuronCores (SPMD data-parallel).

Strategy:
  - Shard batch (32) -> 4 images per core; replicate the tiny bit-plane
    weights/scales on every core. No collectives needed (forward only).
  - Host precomputes the integer conv weights
        W_int[o,i,kh,kw] = sum_b (pweight-nweight)[...,b] * 2^(3-b)   (exact, in [-15,15])
    plus fused scale (scale/15) and bias vectors; bf16 is exact for
    ints <= 15, so the only precision loss is x (bf16) and the fp16
    output store (~1.6e-3 max-rel combined).
  - Host pre-pads each image into the SBUF layout the matmuls read
    ([128, 59*114] bf16: partitions 0:64 = half A = padded rows 0..57,
    partitions 64:128 = half B = padded rows 55..112+zero, row-flattened
    at stride 114, zero pad columns included).
  - On device each half is staged TWICE per image: tile xpA = [half A;
    half A shifted left 1 col], tile xpB likewise for half B. A 3x3 conv
    tap pair (kh,0)+(kh,1) then becomes ONE K=128 matmul against xpA
    (top 64 contraction rows = first tap, bottom 64 = second tap), and
    the two halves' matmuls run CONCURRENTLY as M=64 column-group tiles
    of the 128x128 PE array (tile_position (0,0) / (0,64) - measured
    ~3ns stagger). 9 taps/tile collapse to 6 matmul slots: 3 tap-pair
    slots + 3 single-tap slots (kw=2 column, bottom rows zero-weighted).
  - Epilogue on ACT: out = psum*(scale/15) + bias, cast to fp16; stores
    stream out per 512-col tile. All DMAs are issued in ~1KB-per-
    partition column chunks so descriptors rotate across partitions
    (per-partition SBUF port serializes big descriptors; tiny 448B ones
    are descriptor-rate-bound). Loads ride the gpsimd + vector queues,
    stores the sync queue.
  - Host reassembles the raw [128, 56*114] fp16 tiles to NCHW f32.
"""

import numpy as np
import ml_dtypes

B, C, H, W = 32, 64, 112, 112
NB = 4
CORES = 8
BPC = B // CORES  # images per core

WP = H + 2  # padded width/height = 114
HALF = H // 2  # 56 output rows per position-group
XC_DATA = 58 * WP  # 6612 data columns per partition block
XC = 59 * WP  # + one zero row (junk-column tap reads run past the data)
OUTC = HALF * WP  # 6384 output columns per group

# N-tiles: all >=256 for full PE rate (PSUM bank caps at 512 fp32)
N_TILES = [(i * 512, 512) for i in range(11)] + [(5632, 376), (6008, 376)]
# 6 matmul slots: (rhs column offset, top-tap, bottom-tap or None)
# taps t=(kh,kw) offset kh*114+kw; bottom rows read the shifted copy (+1 col)
SLOTS = [
    (0, 0, 1),      # pair (0,0)+(0,1)
    (114, 3, 4),    # pair (1,0)+(1,1)
    (228, 6, 7),    # pair (2,0)+(2,1)
    (2, 2, None),   # single (0,2)
    (116, 5, None), # single (1,2)
    (230, 8, None), # single (2,2)
]

LOAD_CHUNK = 2048  # elements per DMA chunk (~4KB/partition descriptors)
XCL = 6616  # last column any matmul reads (+1); loads stop here, not XC

_CACHE = {}


def _build():
    if "nc" in _CACHE:
        return _CACHE["nc"]
    import concourse.bacc as bacc
    import concourse.mybir as mybir
    from concourse import tile

    f32 = mybir.dt.float32
    f16 = mybir.dt.float16
    bf16 = mybir.dt.bfloat16

    nc = bacc.Bacc("TRN2", target_bir_lowering=False, debug=False, num_devices=CORES)

    xp_d = nc.dram_tensor("xp", [BPC, 128, XC], bf16, kind="ExternalInput").ap()
    wp_d = nc.dram_tensor("wp6", [128, 6 * C], bf16, kind="ExternalInput").ap()
    sc_d = nc.dram_tensor("scalev", [128, 1], f32, kind="ExternalInput").ap()
    bi_d = nc.dram_tensor("biasv", [128, 1], f32, kind="ExternalInput").ap()
    y_d = nc.dram_tensor("y", [BPC, 128, OUTC], f16, kind="ExternalOutput").ap()

    with tile.TileContext(nc) as tc:
        with (
            tc.tile_pool(name="consts", bufs=1) as consts,
            tc.tile_pool(name="xpool", bufs=3) as xpool,
            tc.tile_pool(name="opool", bufs=2) as opool,
            tc.tile_pool(name="pspool", bufs=8, space="PSUM") as pspool,
        ):
            wp6 = consts.tile([128, 6 * C], bf16, tag="wp6")
            nc.sync.dma_start(wp6[:], wp_d)
            scale_vec = consts.tile([128, 1], f32, tag="scale_vec")
            bias_vec = consts.tile([128, 1], f32, tag="bias_vec")
            nc.sync.dma_start(scale_vec[:], sc_d)
            nc.sync.dma_start(bias_vec[:], bi_d)

            # ---- image load pipeline ----
            # xpA = [half A, half A shifted left 1], xpB likewise for half B.
            # All loads ride the gpsimd queue (sync carries stores, scalar
            # stays free for ACT epilogues), interleaved so both tiles fill
            # in column lockstep and the matmuls can chase the chunks.
            def load_image(b):
                xA = xpool.tile([128, XC], bf16, tag="xpA", name=f"xpA{b}", bufs=4)
                xB = xpool.tile([128, XC], bf16, tag="xpB", name=f"xpB{b}", bufs=4)
                srcA = xp_d[b, 0:C]
                srcB = xp_d[b, C:128]
                for c0 in range(0, XCL, LOAD_CHUNK):
                    c1 = min(c0 + LOAD_CHUNK, XCL)
                    c1s = min(c1, XCL - 1)
                    for xt, src in ((xA, srcA), (xB, srcB)):
                        nc.gpsimd.dma_start(xt[0:C, c0:c1], src[:, c0:c1])
                        if c0 < c1s:
                            nc.gpsimd.dma_start(
                                xt[C:128, c0:c1s], src[:, c0 + 1 : c1s + 1]
                            )
                return xA, xB

            img_next = load_image(0)
            img_next2 = load_image(1)

            # ---- main conv loop ----
            for b in range(BPC):
                xpA, xpB = img_next
                img_next = img_next2
                img_next2 = load_image(b + 2) if b + 2 < BPC else None

                outb = opool.tile([128, OUTC], f16, tag="outb")
                for n0, nt in N_TILES:
                    ps = pspool.tile([128, 512], f32, tag="ps")
                    for s, (off, _t, _tb) in enumerate(SLOTS):
                        w_s = wp6[:, s * C : (s + 1) * C]
                        nc.tensor.matmul(
                            ps[0:C, 0:nt],
                            w_s,
                            xpA[:, n0 + off : n0 + off + nt],
                            start=(s == 0),
                            stop=(s == len(SLOTS) - 1),
                            tile_position=(0, 0),
                        )
                        nc.tensor.matmul(
                            ps[C:128, 0:nt],
                            w_s,
                            xpB[:, n0 + off : n0 + off + nt],
                            start=(s == 0),
                            stop=(s == len(SLOTS) - 1),
                            tile_position=(0, 64),
                        )
                    nc.scalar.activation(
                        outb[:, n0 : n0 + nt],
                        ps[:, 0:nt],
                        mybir.ActivationFunctionType.Identity,
                        bias=bias_vec[:],
                        scale=scale_vec[:],
                    )
                    nc.sync.dma_start(y_d[b, :, n0 : n0 + nt], outb[:, n0 : n0 + nt])

    nc.compile()
    _CACHE["nc"] = nc
    return nc


def _prep_inputs(inputs):
    x = np.asarray(inputs["x"], dtype=np.float32)
    pw = np.asarray(inputs["pweight"], np.float32)
    nw = np.asarray(inputs["nweight"], np.float32)
    pb = np.asarray(inputs["pbias"], np.float32)
    nb = np.asarray(inputs["nbias"], np.float32)
    scale = np.asarray(inputs["scale"], np.float32)[0]
    bscale = np.asarray(inputs["biasscale"], np.float32)[0]

    exps2 = np.array([8.0, 4.0, 2.0, 1.0], np.float32)
    wint = ((pw - nw) * exps2).sum(-1)  # [O, I, 3, 3], exact ints in [-15, 15]
    bias = ((pb - nb) * exps2).sum(-1) * (bscale / 15.0)  # [O]

    # slot-packed stationary operands: [k=128, slot, m=64]
    wp6 = np.zeros((128, 6, C), np.float32)
    for s, (_off, t_top, t_bot) in enumerate(SLOTS):
        kh, kw = divmod(t_top, 3)
        wp6[0:C, s] = wint[:, :, kh, kw].T
        if t_bot is not None:
            kh, kw = divmod(t_bot, 3)
            wp6[C:128, s] = wint[:, :, kh, kw].T
    wp6 = wp6.reshape(128, 6 * C).astype(ml_dtypes.bfloat16)

    scale_vec = np.full((128, 1), scale / 15.0, np.float32)
    bias_vec = np.concatenate([bias, bias]).reshape(128, 1).astype(np.float32)

    # padded SBUF image layout
    xpad = np.zeros((B, 128, XC), dtype=np.float32)
    v = xpad[:, :, :XC_DATA].reshape(B, 128, 58, WP)
    # half A: padded rows 0..57 hold image rows -1..56 (row r = image row r-1)
    v[:, 0:C, 1:58, 1 : 1 + W] = x[:, :, 0:57, :]
    # half B: rows 0..56 hold image rows 55..111, row 57 stays zero
    v[:, C:128, 0:57, 1 : 1 + W] = x[:, :, 55:112, :]
    xpad16 = xpad.astype(ml_dtypes.bfloat16)

    shared = {"wp6": wp6, "scalev": scale_vec, "biasv": bias_vec}
    return [
        dict(shared, xp=np.ascontiguousarray(xpad16[c * BPC : (c + 1) * BPC]))
        for c in range(CORES)
    ]


def _assemble(results):
    """Raw [BPC, 128, 6384] fp16 per core -> [B, 64, 112, 112] f32."""
    out = np.empty((B, C, H, W), dtype=np.float32)
    for c in range(CORES):
        raw = np.asarray(results[c]["y"], dtype=np.float32).reshape(
            BPC, 128, HALF, WP
        )
        out[c * BPC : (c + 1) * BPC, :, 0:HALF, :] = raw[:, 0:C, :, 0:W]
        out[c * BPC : (c + 1) * BPC, :, HALF:H, :] = raw[:, C:128, :, 0:W]
    return out


def _run(inputs, trace=False):
    from concourse.bass_utils import run_bass_kernel_spmd

    nc = _build()
    in_maps = _prep_inputs(inputs)
    last_err = None
    for attempt in range(3):
        try:
            res = run_bass_kernel_spmd(
                nc, in_maps, core_ids=list(range(CORES)), trace=trace
            )
            return _assemble(res.results), res.exec_time_ns
        except Exception as e:  # transient NRT_EXEC_UNIT_UNRECOVERABLE recovers on retry
            last_err = e
            import time

            time.sleep(10)
    raise last_err


def kernel(**inputs) -> np.ndarray:
    out, _ = _run(inputs)
    return out
